# revision 13
# baseline (speedup 1.0000x reference)
"""Bass/Trainium2 kernel for nn_CustomConvWithExtra — v4.

Reference computation (B=32, CIN=COUT=64, H=W=128, K=3, FES=3):
  main = conv3x3(x, conv_w, pad=1) + conv_b
  extra = grouped_conv3x3(broadcast(extra_inputs), extra_w, pad=1) + extra_b
  out = main + extra

The "extra" path's input is spatially constant per (sample, channel), so
it collapses to 9 border-case scalars per (sample, cout) precomputed on
the host and folded into the PSUM->SBUF epilogue as a positional add-map.
The device does the dense 3x3 conv as 9 shifted matmuls accumulated in
PSUM.

Design:
  * bf16 transport: x / weights / output move through HBM as bf16
    (host converts), halving DMA traffic; bf16 matmul streams 1 col/cyc.
    Accumulation stays fp32 in PSUM; the host upcasts the output.
  * x is host-packed into the exact padded SBUF free-dim layout
    [1 + (H+2)*129 + 3] with all halo/pad zeros included, so the x
    load is one fully-contiguous 33.5KB-per-partition DMA per sample.
  * 4-quadrant PE concurrency: two bands in flight.  Even band b:
    sample A (rhs partitions 0-63) -> PSUM 0-63 [tile (0,0)], sample B
    -> PSUM 64-127 [tile (64,64)].  Odd band b+1: A -> PSUM 64-127
    [tile (0,64)], B -> PSUM 0-63 [tile (64,0)].  All four 64x64
    quadrants stream concurrently = full 128x128 array at 1 col/cycle.
  * The parity swap keeps every epilogue op partition-aligned: odd
    bands add a partition-swapped eadd map ([B|A] instead of [A|B]).
  * Output staging is parity-pure: all even bands accumulate compactly
    (pad column stripped) into one SBUF chunk, odd bands into another,
    drained by a handful of large fully-contiguous DMAs per pair (a
    per-band scheme's 181 small DMAs serialized ~111us on HWDGE
    dispatch).  The host re-interleaves bands into NCHW.
  * DMA topology (v7).  Three hard-won facts drive it: (1) the HWDGE
    ring processes descriptors FIFO with a small active window, so
    UNGATED chunks in dispatch order deliver front-to-back at full
    queue rate — explicit WAW chaining (v4/v5) costs ~2us completion-
    semaphore latency per chunk and starves the PE; (2) HWDGE
    completion semaphores are a GLOBAL pool of 8 shared by both HW
    queues, round-robin per DMA, and waiters use cumulative
    thresholds — so a matmul's "my x chunk arrived" wait also counts
    drain increments that share its semaphore, falsely serializing
    compute on drain transfers (v6: 4.1us PE stall + HAM re-throttle);
    (3) SWDGE (gpsimd) uses a separate semaphore set.  Hence: x rides
    BOTH HWDGE queues (sample A on Sync, sample B on Scalar) as
    ungated size-graduated ladders; ALL mid-kernel drains + eadd ride
    the GpSimd SWDGE queue (no shared counters with x); only the
    final drains (nothing waits after them) ride Sync/Scalar.  ~40
    warmup matmuls on zeroed SBUF ramp the PE clock (HAM gate) and
    cover the first x chunk's semaphore latency.

Sharding: data-parallel over batch, 4 samples per core x 8 cores.
"""

import numpy as np

import concourse.bass as bass
import concourse.mybir as mybir
from concourse.tile import TileContext
from concourse.bass_utils import run_bass_kernel_spmd

N_CORES = 8
B, CIN, COUT, FES, H, W, KK = 32, 64, 64, 3, 128, 128, 3
BL = B // N_CORES          # samples per core
NPAIR = BL // 2            # sample pairs per core
RSTRIDE = 129              # padded row stride (W + 1 pad col)
XFREE = 1 + (H + 2) * RSTRIDE + 3   # 16774: lead zero + 130 padded rows + tail
RB = 3                     # output rows per band (PSUM tile)
NBAND = (H + RB - 1) // RB  # 43 bands; last band has 2 rows
NMAX = RB * RSTRIDE        # 387 fp32 <= 512 (one PSUM bank)
NEV = (NBAND + 1) // 2      # 22 even bands (incl. short band 42)
NOD = NBAND // 2            # 21 odd bands
SLOT = RB * W               # 384 compact cols per band slot
EV_FREE = NEV * SLOT        # 8448
OD_FREE = NOD * SLOT        # 8064

# eadd free-dim offsets: band 0 -> first pattern, 1..41 -> mid, 42 -> last
_E_FIRST, _E_MID, _E_LAST = 0, NMAX, 2 * NMAX
EADD_FREE = 2 * NMAX + (H - RB * (NBAND - 1)) * RSTRIDE  # 387+387+258 = 1032


def _band_rows(b):
    i0 = b * RB
    return i0, min(RB, H - i0)


def split_sync_waits(nc):
    """This toolchain's walrus accepts only ONE sync-wait per instruction.
    Hoist extra waits onto single-wait NoOps inserted just before, on the
    same engine (same queue => same semantics)."""
    for func in nc.m.functions:
        for block in func.blocks:
            out = []
            changed = False
            for inst in block.instructions:
                si = inst.sync_info
                waits = list(si.on_wait) if (si and si.on_wait) else []
                if len(waits) > 1:
                    changed = True
                    for k, w in enumerate(waits[:-1]):
                        nop = mybir.InstNoOp(
                            name=f"{inst.name}-sw{k}",
                            engine=inst.engine,
                            sync_info=mybir.SyncInfo(on_wait=[w], on_update=[]),
                            bass_nofuse=True,
                        )
                        nc.register_instruction(nop, overwrite=True)
                        out.append(nop)
                    inst.sync_info = mybir.SyncInfo(
                        on_wait=[waits[-1]], on_update=list(si.on_update or [])
                    )
                out.append(inst)
            if changed:
                block.instructions = out


def build_program():
    f32 = mybir.dt.float32
    bf16 = mybir.dt.bfloat16
    nc = bass.Bass("TRN2", target_bir_lowering=False, debug=False,
                   num_devices=N_CORES)
    # host-packed padded bf16 x: [BL, CIN, XFREE]
    x = nc.dram_tensor("x", [BL, CIN, XFREE], bf16, kind="ExternalInput")
    wt = nc.dram_tensor("wt", [128, 9 * COUT], bf16, kind="ExternalInput")
    # eadd maps per sample pair: normal [A|B] (first/mid/last patterns) and
    # partition-swapped [B|A] (mid pattern only, for odd bands)
    eadd = nc.dram_tensor("eadd", [NPAIR, 128, EADD_FREE], bf16,
                          kind="ExternalInput")
    eswp = nc.dram_tensor("eswp", [NPAIR, 128, NMAX], bf16,
                          kind="ExternalInput")
    # compact band-major outputs: even bands of sample s in out_ev[s];
    # odd bands of sample s in out_od[s ^ 1] (pair-swapped partition halves)
    out_ev = nc.dram_tensor("out_ev", [BL, COUT, EV_FREE], bf16,
                            kind="ExternalOutput")
    out_od = nc.dram_tensor("out_od", [BL, COUT, OD_FREE], bf16,
                            kind="ExternalOutput")

    # x ladders: size-graduated ungated chunks (small first so band 0's
    # completion sem fires early), one DMA per 64-partition sample half.
    # Each queue's FIFO paces pair 1's descriptors behind pair 0's.
    # Boundaries in padded-row units (flat = 1 + r*129).
    P0_ROWS = [0, 8, 18, 30, 44, 60, 78, 98, 130]
    P1_ROWS = [0, 20, 50, 90, 130]

    def _row_flat(r):
        return 1 + r * RSTRIDE

    with TileContext(nc) as tc:
        with (
            tc.tile_pool(name="wp", bufs=1) as wp,
            tc.tile_pool(name="xp", bufs=2) as xp,
            tc.tile_pool(name="ep", bufs=2) as ep,
            tc.tile_pool(name="op", bufs=2) as op,
            tc.tile_pool(name="pp", bufs=8, space="PSUM") as pp,
        ):
            # PE warmup: no-dep matmuls on zeroed SBUF keep the PE busy
            # (ramping the HAM clock gate) until the weights + first x
            # chunks arrive.  The warmup PSUM tile joins the band
            # rotation (its reuse trivially waits on the warmup).
            wu_sb = wp.tile([64, 192], bf16)
            nc.vector.memset(wu_sb[:], 0.0)
            wu_ps = pp.tile([128, 512], f32, tag="ps", name="wu_ps")
            for _ in range(40):
                nc.tensor.matmul(wu_ps[0:64, 0:128], wu_sb[:, 0:64],
                                 wu_sb[:, 64:192], start=True, stop=True)

            # wt leads the Scalar ring: completes first, so its
            # completion sem (gating every LDWEIGHTS) fires early.
            wt_sb = wp.tile([128, 9 * COUT], bf16)
            nc.scalar.dma_start(out=wt_sb[:], in_=wt[:])

            xts, ets, ess = [], [], []
            for sp in range(NPAIR):
                xt = xp.tile([128, XFREE], bf16, tag="xt", name=f"xt{sp}")
                xts.append(xt)
                rows = P0_ROWS if sp == 0 else P1_ROWS
                for k in range(len(rows) - 1):
                    c0 = 0 if k == 0 else _row_flat(rows[k])
                    c1 = XFREE if k == len(rows) - 2 else _row_flat(rows[k + 1])
                    nc.sync.dma_start(out=xt[0:64, c0:c1],
                                      in_=x[2 * sp, :, c0:c1])
                    nc.scalar.dma_start(out=xt[64:128, c0:c1],
                                        in_=x[2 * sp + 1, :, c0:c1])
                et = ep.tile([128, EADD_FREE], bf16, tag="et",
                             name=f"et{sp}")
                nc.gpsimd.dma_start(out=et[:], in_=eadd[sp])
                ets.append(et)
                es = ep.tile([128, NMAX], bf16, tag="es", name=f"es{sp}")
                nc.gpsimd.dma_start(out=es[:], in_=eswp[sp])
                ess.append(es)

            for sp in range(NPAIR):
                xt, et, es = xts[sp], ets[sp], ess[sp]

                ot_ev = op.tile([128, EV_FREE], bf16, tag="ot_ev")
                ot_od = op.tile([128, OD_FREE], bf16, tag="ot_od")
                # band 42 fills only 256 of its 384-col slot; zero the tail
                # so the chunk DMA never reads uninitialized SBUF
                nc.vector.memset(ot_ev[:, EV_FREE - SLOT + 2 * W:EV_FREE], 0.0)

                # progressive output drains on the GpSimd SWDGE queue
                # (separate semaphore set — HWDGE drains would falsely
                # serialize matmul x-waits on shared completion
                # counters): (trigger bs, ev slots [e0,e1), od slots
                # [o0,o1)); at trigger bs all bands < bs are complete.
                # The bs=42 drains ride Sync/Scalar: x is long done, no
                # waiter follows them, and they run in parallel with
                # the SWDGE tail.
                drains = {8: (0, 4, 0, 4), 14: (4, 7, 4, 7),
                          20: (7, 10, 7, 10), 26: (10, 13, 10, 13),
                          32: (13, 16, 13, 16), 38: (16, 19, 16, 19),
                          42: (19, 21, 19, 21)}
                for bs in range(0, NBAND, 2):
                    dr = drains.get(bs)
                    if dr is not None:
                        e0, e1, o0, o1 = dr
                        qe = nc.sync if bs == 42 else nc.gpsimd
                        qo = nc.scalar if bs == 42 else nc.gpsimd
                        qe.dma_start(
                            out=out_ev[2 * sp:2 * sp + 2, :,
                                       e0 * SLOT:e1 * SLOT]
                            .rearrange("s c f -> (s c) f"),
                            in_=ot_ev[:, e0 * SLOT:e1 * SLOT])
                        qo.dma_start(
                            out=out_od[2 * sp:2 * sp + 2, :,
                                       o0 * SLOT:o1 * SLOT]
                            .rearrange("s c f -> (s c) f"),
                            in_=ot_od[:, o0 * SLOT:o1 * SLOT])
                    bands = [b for b in (bs, bs + 1) if b < NBAND]
                    pss = {}
                    for b in bands:
                        ps = pp.tile([128, 512], f32, tag="ps",
                                     name=f"ps{b}")
                        pss[b] = ps
                    for tap in range(9):
                        di, dj = divmod(tap, 3)
                        st, sp_ = (tap == 0), (tap == 8)
                        wA = wt_sb[0:64, tap * COUT:(tap + 1) * COUT]
                        wB = wt_sb[64:128, tap * COUT:(tap + 1) * COUT]
                        for b in bands:
                            i0, rb = _band_rows(b)
                            n = rb * RSTRIDE
                            off = (i0 + di) * RSTRIDE + dj
                            ps = pss[b]
                            if b % 2 == 0:
                                # A -> psum 0:64, B -> psum 64:128
                                nc.tensor.matmul(
                                    ps[0:64, 0:n], wA,
                                    xt[0:64, off:off + n],
                                    start=st, stop=sp_)
                                nc.tensor.matmul(
                                    ps[64:128, 0:n], wB,
                                    xt[64:128, off:off + n],
                                    start=st, stop=sp_)
                            else:
                                # A -> psum 64:128, B -> psum 0:64
                                nc.tensor.matmul(
                                    ps[64:128, 0:n], wA,
                                    xt[0:64, off:off + n],
                                    start=st, stop=sp_)
                                nc.tensor.matmul(
                                    ps[0:64, 0:n], wB,
                                    xt[64:128, off:off + n],
                                    start=st, stop=sp_)
                    for b in bands:
                        i0, rb = _band_rows(b)
                        n = rb * RSTRIDE
                        k = b // 2
                        ps3 = pss[b][:, 0:n].rearrange(
                            "p (r c) -> p r c", c=RSTRIDE)[:, :, 0:W]
                        if b % 2 == 0:
                            eo = (_E_FIRST if b == 0 else
                                  (_E_LAST if b == NBAND - 1 else _E_MID))
                            et3 = et[:, eo:eo + n].rearrange(
                                "p (r c) -> p r c", c=RSTRIDE)[:, :, 0:W]
                            ot3 = ot_ev[:, k * SLOT:k * SLOT + rb * W]\
                                .rearrange("p (r c) -> p r c", c=W)
                            nc.vector.tensor_add(ot3, ps3, et3)
                        else:
                            es3 = es[:, 0:n].rearrange(
                                "p (r c) -> p r c", c=RSTRIDE)[:, :, 0:W]
                            ot3 = ot_od[:, k * SLOT:k * SLOT + rb * W]\
                                .rearrange("p (r c) -> p r c", c=W)
                            nc.vector.tensor_add(ot3, ps3, es3)
                # final segment: ev slot 21 (band 42, includes the zeroed
                # tail); od is fully drained by the bs=42 trigger.
                nc.sync.dma_start(
                    out=out_ev[2 * sp:2 * sp + 2, :, 21 * SLOT:]
                    .rearrange("s c f -> (s c) f"),
                    in_=ot_ev[:, 21 * SLOT:])

    split_sync_waits(nc)
    return nc


_PROGRAM = None


def _get_program():
    global _PROGRAM
    if _PROGRAM is None:
        _PROGRAM = build_program()
    return _PROGRAM


def host_prepack(x, extra_inputs, conv_w, conv_b, extra_w, extra_b):
    """Fold weights/biases/extra-path into device-ready arrays."""
    bf16 = mybir.dt.np(mybir.dt.bfloat16)

    # padded bf16 x layout: x[s, ci, 1 + (r+1)*129 + c] = x[s, ci, r, c]
    xp = np.zeros((B, CIN, XFREE), dtype=bf16)
    xv = xp[:, :, 1 + RSTRIDE:1 + (H + 1) * RSTRIDE].reshape(
        B, CIN, H, RSTRIDE)
    xv[:, :, :, 0:W] = x.astype(bf16)

    # wt[ci, tap*64+co] = conv_w[co, ci, di, dj], tap = di*3+dj; both halves
    wt_half = np.ascontiguousarray(
        conv_w.transpose(1, 2, 3, 0)).reshape(CIN, 9 * COUT)
    wt = np.concatenate([wt_half, wt_half], axis=0).astype(bf16)

    # border-case extra values: E[s, rowclass, colclass, co]
    row_sel = [slice(1, 3), slice(0, 3), slice(0, 2)]   # top, mid, bot
    col_sel = [slice(1, 3), slice(0, 3), slice(0, 2)]   # left, mid, right
    wsum = np.zeros((3, 3, COUT, FES), np.float32)
    for rc in range(3):
        for cc in range(3):
            wsum[rc, cc] = extra_w[:, :, row_sel[rc], col_sel[cc]].sum((2, 3))
    ein = extra_inputs.reshape(B, COUT, FES)
    e9 = np.einsum('scf,rkcf->srkc', ein, wsum)
    e9 = e9 + (extra_b + conv_b)[None, None, None, :]   # [s, rc, cc, co]

    # positional row patterns at stride 129 (last slot = pad, value 0)
    def row_vec(s, rc):
        v = np.zeros((COUT, RSTRIDE), np.float32)
        v[:, 0] = e9[s, rc, 0]
        v[:, 1:W - 1] = e9[s, rc, 1][:, None]
        v[:, W - 1] = e9[s, rc, 2]
        return v

    eadd = np.zeros((B, COUT, EADD_FREE), np.float32)
    for s in range(B):
        top, mid, bot = row_vec(s, 0), row_vec(s, 1), row_vec(s, 2)
        eadd[s, :, 0:NMAX] = np.concatenate([top, mid, mid], 1)
        eadd[s, :, NMAX:2 * NMAX] = np.concatenate([mid, mid, mid], 1)
        eadd[s, :, 2 * NMAX:] = np.concatenate([mid, bot], 1)
    return xp, wt, eadd


# row indices of even-band rows (63) and odd-band rows (63) in the image
_EV_ROWS = (np.arange(NEV - 1)[:, None] * 2 * RB + np.arange(RB)).ravel()
_OD_ROWS = (np.arange(NOD)[:, None] * 2 * RB + RB + np.arange(RB)).ravel()


def _assemble(out_ev, out_od):
    """Re-interleave compact band-major bf16 chunks into NCHW fp32."""
    out = np.empty((B, COUT, H, W), np.float32)
    # even bands 0..40 (21 full slots), band 42 (rows 126,127) special
    ev = out_ev[:, :, :(NEV - 1) * SLOT].reshape(B, COUT, -1, W)
    out[:, :, _EV_ROWS, :] = ev
    b42 = out_ev[:, :, (NEV - 1) * SLOT:(NEV - 1) * SLOT + 2 * W].reshape(
        B, COUT, 2, W)
    out[:, :, H - 2:H, :] = b42
    # odd bands, pair-swapped samples: sample s odd bands live in
    # out_od[s ^ 1]
    swap = np.arange(B) ^ 1
    od = out_od[swap][:, :, :].reshape(B, COUT, -1, W)
    out[:, :, _OD_ROWS, :] = od
    return out


def kernel(x, extra_inputs, conv_w, conv_b, extra_w, extra_b):
    x = np.asarray(x, np.float32)
    xp, wt, eadd = host_prepack(
        x, np.asarray(extra_inputs, np.float32),
        np.asarray(conv_w, np.float32), np.asarray(conv_b, np.float32),
        np.asarray(extra_w, np.float32), np.asarray(extra_b, np.float32))

    nc = _get_program()
    bf16 = mybir.dt.np(mybir.dt.bfloat16)
    in_maps = []
    for k in range(N_CORES):
        s0 = k * BL
        epair = np.stack(
            [np.concatenate([eadd[s0 + 2 * p], eadd[s0 + 2 * p + 1]], axis=0)
             for p in range(NPAIR)])
        # swapped [B|A], mid pattern only (odd bands are never first/last)
        eswp = np.stack(
            [np.concatenate([eadd[s0 + 2 * p + 1, :, NMAX:2 * NMAX],
                             eadd[s0 + 2 * p, :, NMAX:2 * NMAX]], axis=0)
             for p in range(NPAIR)])
        in_maps.append({
            "x": xp[s0:s0 + BL],
            "wt": wt,
            "eadd": np.ascontiguousarray(epair.astype(bf16)),
            "eswp": np.ascontiguousarray(eswp.astype(bf16)),
        })
    res = run_bass_kernel_spmd(nc, in_maps, list(range(N_CORES)))
    global _LAST_RESULTS
    _LAST_RESULTS = res
    out_ev = np.concatenate(
        [res.results[k]["out_ev"] for k in range(N_CORES)], axis=0)
    out_od = np.concatenate(
        [res.results[k]["out_od"] for k in range(N_CORES)], axis=0)
    return _assemble(out_ev, out_od)


_LAST_RESULTS = None  # BassKernelResults of the most recent run (test harness)



# revision 16
# speedup vs baseline: 1.0625x; 1.0625x over previous
"""Bass/Trainium2 kernel for nn_CustomConvWithExtra — v4.

Reference computation (B=32, CIN=COUT=64, H=W=128, K=3, FES=3):
  main = conv3x3(x, conv_w, pad=1) + conv_b
  extra = grouped_conv3x3(broadcast(extra_inputs), extra_w, pad=1) + extra_b
  out = main + extra

The "extra" path's input is spatially constant per (sample, channel), so
it collapses to 9 border-case scalars per (sample, cout) precomputed on
the host and folded into the PSUM->SBUF epilogue as a positional add-map.
The device does the dense 3x3 conv as 9 shifted matmuls accumulated in
PSUM.

Design:
  * bf16 transport: x / weights / output move through HBM as bf16
    (host converts), halving DMA traffic; bf16 matmul streams 1 col/cyc.
    Accumulation stays fp32 in PSUM; the host upcasts the output.
  * x is host-packed into the exact padded SBUF free-dim layout
    [1 + (H+2)*129 + 3] with all halo/pad zeros included, so the x
    load is one fully-contiguous 33.5KB-per-partition DMA per sample.
  * 4-quadrant PE concurrency: two bands in flight.  Even band b:
    sample A (rhs partitions 0-63) -> PSUM 0-63 [tile (0,0)], sample B
    -> PSUM 64-127 [tile (64,64)].  Odd band b+1: A -> PSUM 64-127
    [tile (0,64)], B -> PSUM 0-63 [tile (64,0)].  All four 64x64
    quadrants stream concurrently = full 128x128 array at 1 col/cycle.
  * The parity swap keeps every epilogue op partition-aligned: odd
    bands add a partition-swapped eadd map ([B|A] instead of [A|B]).
  * Output staging is parity-pure: all even bands accumulate compactly
    (pad column stripped) into one SBUF chunk, odd bands into another,
    drained by a handful of large fully-contiguous DMAs per pair (a
    per-band scheme's 181 small DMAs serialized ~111us on HWDGE
    dispatch).  The host re-interleaves bands into NCHW.
  * DMA topology (v8).  Hard-won facts: (1) the HWDGE ring processes
    descriptors FIFO with a small active window, so UNGATED chunks in
    dispatch order deliver front-to-back at full queue rate —
    explicit WAW chaining (v4/v5) costs ~2us completion-semaphore
    latency per chunk and starves the PE; (2) HWDGE completion
    semaphores are a GLOBAL pool of 8 shared by both HW queues,
    assigned per-DMA in SCHEDULE order, and waiters use cumulative
    thresholds — so any x-chunk waiter whose semaphore is also
    incremented by an earlier-scheduled drain falsely serializes
    compute on that drain's transfer (v6: 4.1us PE stall + HAM
    re-throttle); (3) SWDGE is too slow for bulk output (~120GB/s,
    v7 regression).  Hence: ALL x on the Sync queue as an ungated
    size-graduated ladder, emitted under tc.high_priority() so the
    scheduler places every x DMA before every drain (no drain can
    enter an x-waiter's threshold); ALL mid-kernel drains on Scalar;
    final drains (no waiters after them) split Sync/Scalar; eadd on
    GpSimd SWDGE.  ~40 warmup matmuls on zeroed SBUF ramp the PE
    clock (HAM gate) and cover the first x chunk's semaphore latency.

Sharding: data-parallel over batch, 4 samples per core x 8 cores.
"""

import numpy as np

import concourse.bass as bass
import concourse.mybir as mybir
from concourse.tile import TileContext
from concourse.bass_utils import run_bass_kernel_spmd

N_CORES = 8
B, CIN, COUT, FES, H, W, KK = 32, 64, 64, 3, 128, 128, 3
BL = B // N_CORES          # samples per core
NPAIR = BL // 2            # sample pairs per core
RSTRIDE = 129              # padded row stride (W + 1 pad col)
XFREE = 1 + (H + 2) * RSTRIDE + 3   # 16774: lead zero + 130 padded rows + tail
RB = 3                     # output rows per band (PSUM tile)
NBAND = (H + RB - 1) // RB  # 43 bands; last band has 2 rows
NMAX = RB * RSTRIDE        # 387 fp32 <= 512 (one PSUM bank)
NEV = (NBAND + 1) // 2      # 22 even bands (incl. short band 42)
NOD = NBAND // 2            # 21 odd bands
SLOT = RB * W               # 384 compact cols per band slot
EV_FREE = NEV * SLOT        # 8448
OD_FREE = NOD * SLOT        # 8064

# eadd free-dim offsets: band 0 -> first pattern, 1..41 -> mid, 42 -> last
_E_FIRST, _E_MID, _E_LAST = 0, NMAX, 2 * NMAX
EADD_FREE = 2 * NMAX + (H - RB * (NBAND - 1)) * RSTRIDE  # 387+387+258 = 1032


def _band_rows(b):
    i0 = b * RB
    return i0, min(RB, H - i0)


def split_sync_waits(nc):
    """This toolchain's walrus accepts only ONE sync-wait per instruction.
    Hoist extra waits onto single-wait NoOps inserted just before, on the
    same engine (same queue => same semantics)."""
    for func in nc.m.functions:
        for block in func.blocks:
            out = []
            changed = False
            for inst in block.instructions:
                si = inst.sync_info
                waits = list(si.on_wait) if (si and si.on_wait) else []
                if len(waits) > 1:
                    changed = True
                    for k, w in enumerate(waits[:-1]):
                        nop = mybir.InstNoOp(
                            name=f"{inst.name}-sw{k}",
                            engine=inst.engine,
                            sync_info=mybir.SyncInfo(on_wait=[w], on_update=[]),
                            bass_nofuse=True,
                        )
                        nc.register_instruction(nop, overwrite=True)
                        out.append(nop)
                    inst.sync_info = mybir.SyncInfo(
                        on_wait=[waits[-1]], on_update=list(si.on_update or [])
                    )
                out.append(inst)
            if changed:
                block.instructions = out


def build_program():
    f32 = mybir.dt.float32
    bf16 = mybir.dt.bfloat16
    nc = bass.Bass("TRN2", target_bir_lowering=False, debug=False,
                   num_devices=N_CORES)
    # host-packed padded bf16 x: [BL, CIN, XFREE]
    x = nc.dram_tensor("x", [BL, CIN, XFREE], bf16, kind="ExternalInput")
    wt = nc.dram_tensor("wt", [128, 9 * COUT], bf16, kind="ExternalInput")
    # eadd maps per sample pair: normal [A|B] (first/mid/last patterns) and
    # partition-swapped [B|A] (mid pattern only, for odd bands)
    eadd = nc.dram_tensor("eadd", [NPAIR, 128, EADD_FREE], bf16,
                          kind="ExternalInput")
    eswp = nc.dram_tensor("eswp", [NPAIR, 128, NMAX], bf16,
                          kind="ExternalInput")
    # compact band-major outputs: even bands of sample s in out_ev[s];
    # odd bands of sample s in out_od[s ^ 1] (pair-swapped partition halves)
    out_ev = nc.dram_tensor("out_ev", [BL, COUT, EV_FREE], bf16,
                            kind="ExternalOutput")
    out_od = nc.dram_tensor("out_od", [BL, COUT, OD_FREE], bf16,
                            kind="ExternalOutput")

    # x ladders: size-graduated ungated chunks (small first so band 0's
    # completion sem fires early), one DMA per 64-partition sample half.
    # Each queue's FIFO paces pair 1's descriptors behind pair 0's.
    # Boundaries in padded-row units (flat = 1 + r*129).
    P0_ROWS = [0, 8, 18, 30, 44, 60, 78, 98, 130]
    P1_ROWS = [0, 20, 50, 90, 130]

    def _row_flat(r):
        return 1 + r * RSTRIDE

    with TileContext(nc) as tc:
        with (
            tc.tile_pool(name="wp", bufs=1) as wp,
            tc.tile_pool(name="xp", bufs=2) as xp,
            tc.tile_pool(name="ep", bufs=2) as ep,
            tc.tile_pool(name="op", bufs=2) as op,
            tc.tile_pool(name="pp", bufs=8, space="PSUM") as pp,
        ):
            # PE warmup: no-dep matmuls on zeroed SBUF keep the PE busy
            # (ramping the HAM clock gate) until the weights + first x
            # chunks arrive.  The warmup PSUM tile joins the band
            # rotation (its reuse trivially waits on the warmup).
            wu_sb = wp.tile([64, 192], bf16)
            nc.vector.memset(wu_sb[:], 0.0)
            wu_ps = pp.tile([128, 512], f32, tag="ps", name="wu_ps")
            for _ in range(40):
                nc.tensor.matmul(wu_ps[0:64, 0:128], wu_sb[:, 0:64],
                                 wu_sb[:, 64:192], start=True, stop=True)

            # wt + ALL x chunks are emitted at scheduler priority 0:
            # the Tile scheduler must place every x DMA BEFORE every
            # drain in its schedule, so no x-waiter's cumulative
            # completion-semaphore threshold can include a (slow, late)
            # drain's increments (the v6 false-dependency stall).
            xts, ets, ess = [], [], []
            with tc.high_priority():
                # wt leads the Sync ring: completes first, so its
                # completion sem (gating every LDWEIGHTS) fires early.
                wt_sb = wp.tile([128, 9 * COUT], bf16)
                nc.sync.dma_start(out=wt_sb[:], in_=wt[:])
                for sp in range(NPAIR):
                    xt = xp.tile([128, XFREE], bf16, tag="xt",
                                 name=f"xt{sp}")
                    xts.append(xt)
                    rows = P0_ROWS if sp == 0 else P1_ROWS
                    for k in range(len(rows) - 1):
                        c0 = 0 if k == 0 else _row_flat(rows[k])
                        c1 = (XFREE if k == len(rows) - 2
                              else _row_flat(rows[k + 1]))
                        nc.sync.dma_start(out=xt[0:64, c0:c1],
                                          in_=x[2 * sp, :, c0:c1])
                        nc.sync.dma_start(out=xt[64:128, c0:c1],
                                          in_=x[2 * sp + 1, :, c0:c1])
            for sp in range(NPAIR):
                et = ep.tile([128, EADD_FREE], bf16, tag="et",
                             name=f"et{sp}")
                nc.gpsimd.dma_start(out=et[:], in_=eadd[sp])
                ets.append(et)
                es = ep.tile([128, NMAX], bf16, tag="es", name=f"es{sp}")
                nc.gpsimd.dma_start(out=es[:], in_=eswp[sp])
                ess.append(es)

            for sp in range(NPAIR):
                xt, et, es = xts[sp], ets[sp], ess[sp]

                ot_ev = op.tile([128, EV_FREE], bf16, tag="ot_ev")
                ot_od = op.tile([128, OD_FREE], bf16, tag="ot_od")
                # band 42 fills only 256 of its 384-col slot; zero the tail
                # so the chunk DMA never reads uninitialized SBUF
                nc.vector.memset(ot_ev[:, EV_FREE - SLOT + 2 * W:EV_FREE], 0.0)

                # progressive output drains on the Scalar HWDGE queue
                # (SWDGE measured too slow, ~120GB/s, for 8MB of out);
                # safe against false x-waiter deps because all x DMAs
                # are scheduled first (high_priority above): (trigger
                # bs, ev slots [e0,e1), od slots [o0,o1)); at trigger
                # bs all bands < bs are complete.  The bs=42 ev drain
                # rides Sync (idle by then) for a parallel tail.
                drains = {8: (0, 4, 0, 4), 14: (4, 7, 4, 7),
                          20: (7, 10, 7, 10), 26: (10, 13, 10, 13),
                          32: (13, 16, 13, 16), 38: (16, 19, 16, 19),
                          42: (19, 21, 19, 21)}
                for bs in range(0, NBAND, 2):
                    dr = drains.get(bs)
                    if dr is not None:
                        e0, e1, o0, o1 = dr
                        qe = nc.sync if bs == 42 else nc.scalar
                        qo = nc.scalar
                        qe.dma_start(
                            out=out_ev[2 * sp:2 * sp + 2, :,
                                       e0 * SLOT:e1 * SLOT]
                            .rearrange("s c f -> (s c) f"),
                            in_=ot_ev[:, e0 * SLOT:e1 * SLOT])
                        qo.dma_start(
                            out=out_od[2 * sp:2 * sp + 2, :,
                                       o0 * SLOT:o1 * SLOT]
                            .rearrange("s c f -> (s c) f"),
                            in_=ot_od[:, o0 * SLOT:o1 * SLOT])
                    bands = [b for b in (bs, bs + 1) if b < NBAND]
                    pss = {}
                    for b in bands:
                        ps = pp.tile([128, 512], f32, tag="ps",
                                     name=f"ps{b}")
                        pss[b] = ps
                    for tap in range(9):
                        di, dj = divmod(tap, 3)
                        st, sp_ = (tap == 0), (tap == 8)
                        wA = wt_sb[0:64, tap * COUT:(tap + 1) * COUT]
                        wB = wt_sb[64:128, tap * COUT:(tap + 1) * COUT]
                        for b in bands:
                            i0, rb = _band_rows(b)
                            n = rb * RSTRIDE
                            off = (i0 + di) * RSTRIDE + dj
                            ps = pss[b]
                            if b % 2 == 0:
                                # A -> psum 0:64, B -> psum 64:128
                                nc.tensor.matmul(
                                    ps[0:64, 0:n], wA,
                                    xt[0:64, off:off + n],
                                    start=st, stop=sp_)
                                nc.tensor.matmul(
                                    ps[64:128, 0:n], wB,
                                    xt[64:128, off:off + n],
                                    start=st, stop=sp_)
                            else:
                                # A -> psum 64:128, B -> psum 0:64
                                nc.tensor.matmul(
                                    ps[64:128, 0:n], wA,
                                    xt[0:64, off:off + n],
                                    start=st, stop=sp_)
                                nc.tensor.matmul(
                                    ps[0:64, 0:n], wB,
                                    xt[64:128, off:off + n],
                                    start=st, stop=sp_)
                    for b in bands:
                        i0, rb = _band_rows(b)
                        n = rb * RSTRIDE
                        k = b // 2
                        ps3 = pss[b][:, 0:n].rearrange(
                            "p (r c) -> p r c", c=RSTRIDE)[:, :, 0:W]
                        if b % 2 == 0:
                            eo = (_E_FIRST if b == 0 else
                                  (_E_LAST if b == NBAND - 1 else _E_MID))
                            et3 = et[:, eo:eo + n].rearrange(
                                "p (r c) -> p r c", c=RSTRIDE)[:, :, 0:W]
                            ot3 = ot_ev[:, k * SLOT:k * SLOT + rb * W]\
                                .rearrange("p (r c) -> p r c", c=W)
                            nc.vector.tensor_add(ot3, ps3, et3)
                        else:
                            es3 = es[:, 0:n].rearrange(
                                "p (r c) -> p r c", c=RSTRIDE)[:, :, 0:W]
                            ot3 = ot_od[:, k * SLOT:k * SLOT + rb * W]\
                                .rearrange("p (r c) -> p r c", c=W)
                            nc.vector.tensor_add(ot3, ps3, es3)
                # final segment: ev slot 21 (band 42, includes the zeroed
                # tail); od is fully drained by the bs=42 trigger.
                nc.sync.dma_start(
                    out=out_ev[2 * sp:2 * sp + 2, :, 21 * SLOT:]
                    .rearrange("s c f -> (s c) f"),
                    in_=ot_ev[:, 21 * SLOT:])

    split_sync_waits(nc)
    return nc


_PROGRAM = None


def _get_program():
    global _PROGRAM
    if _PROGRAM is None:
        _PROGRAM = build_program()
    return _PROGRAM


def host_prepack(x, extra_inputs, conv_w, conv_b, extra_w, extra_b):
    """Fold weights/biases/extra-path into device-ready arrays."""
    bf16 = mybir.dt.np(mybir.dt.bfloat16)

    # padded bf16 x layout: x[s, ci, 1 + (r+1)*129 + c] = x[s, ci, r, c]
    xp = np.zeros((B, CIN, XFREE), dtype=bf16)
    xv = xp[:, :, 1 + RSTRIDE:1 + (H + 1) * RSTRIDE].reshape(
        B, CIN, H, RSTRIDE)
    xv[:, :, :, 0:W] = x.astype(bf16)

    # wt[ci, tap*64+co] = conv_w[co, ci, di, dj], tap = di*3+dj; both halves
    wt_half = np.ascontiguousarray(
        conv_w.transpose(1, 2, 3, 0)).reshape(CIN, 9 * COUT)
    wt = np.concatenate([wt_half, wt_half], axis=0).astype(bf16)

    # border-case extra values: E[s, rowclass, colclass, co]
    row_sel = [slice(1, 3), slice(0, 3), slice(0, 2)]   # top, mid, bot
    col_sel = [slice(1, 3), slice(0, 3), slice(0, 2)]   # left, mid, right
    wsum = np.zeros((3, 3, COUT, FES), np.float32)
    for rc in range(3):
        for cc in range(3):
            wsum[rc, cc] = extra_w[:, :, row_sel[rc], col_sel[cc]].sum((2, 3))
    ein = extra_inputs.reshape(B, COUT, FES)
    e9 = np.einsum('scf,rkcf->srkc', ein, wsum)
    e9 = e9 + (extra_b + conv_b)[None, None, None, :]   # [s, rc, cc, co]

    # positional row patterns at stride 129 (last slot = pad, value 0)
    def row_vec(s, rc):
        v = np.zeros((COUT, RSTRIDE), np.float32)
        v[:, 0] = e9[s, rc, 0]
        v[:, 1:W - 1] = e9[s, rc, 1][:, None]
        v[:, W - 1] = e9[s, rc, 2]
        return v

    eadd = np.zeros((B, COUT, EADD_FREE), np.float32)
    for s in range(B):
        top, mid, bot = row_vec(s, 0), row_vec(s, 1), row_vec(s, 2)
        eadd[s, :, 0:NMAX] = np.concatenate([top, mid, mid], 1)
        eadd[s, :, NMAX:2 * NMAX] = np.concatenate([mid, mid, mid], 1)
        eadd[s, :, 2 * NMAX:] = np.concatenate([mid, bot], 1)
    return xp, wt, eadd


# row indices of even-band rows (63) and odd-band rows (63) in the image
_EV_ROWS = (np.arange(NEV - 1)[:, None] * 2 * RB + np.arange(RB)).ravel()
_OD_ROWS = (np.arange(NOD)[:, None] * 2 * RB + RB + np.arange(RB)).ravel()


def _assemble(out_ev, out_od):
    """Re-interleave compact band-major bf16 chunks into NCHW fp32."""
    out = np.empty((B, COUT, H, W), np.float32)
    # even bands 0..40 (21 full slots), band 42 (rows 126,127) special
    ev = out_ev[:, :, :(NEV - 1) * SLOT].reshape(B, COUT, -1, W)
    out[:, :, _EV_ROWS, :] = ev
    b42 = out_ev[:, :, (NEV - 1) * SLOT:(NEV - 1) * SLOT + 2 * W].reshape(
        B, COUT, 2, W)
    out[:, :, H - 2:H, :] = b42
    # odd bands, pair-swapped samples: sample s odd bands live in
    # out_od[s ^ 1]
    swap = np.arange(B) ^ 1
    od = out_od[swap][:, :, :].reshape(B, COUT, -1, W)
    out[:, :, _OD_ROWS, :] = od
    return out


def kernel(x, extra_inputs, conv_w, conv_b, extra_w, extra_b):
    x = np.asarray(x, np.float32)
    xp, wt, eadd = host_prepack(
        x, np.asarray(extra_inputs, np.float32),
        np.asarray(conv_w, np.float32), np.asarray(conv_b, np.float32),
        np.asarray(extra_w, np.float32), np.asarray(extra_b, np.float32))

    nc = _get_program()
    bf16 = mybir.dt.np(mybir.dt.bfloat16)
    in_maps = []
    for k in range(N_CORES):
        s0 = k * BL
        epair = np.stack(
            [np.concatenate([eadd[s0 + 2 * p], eadd[s0 + 2 * p + 1]], axis=0)
             for p in range(NPAIR)])
        # swapped [B|A], mid pattern only (odd bands are never first/last)
        eswp = np.stack(
            [np.concatenate([eadd[s0 + 2 * p + 1, :, NMAX:2 * NMAX],
                             eadd[s0 + 2 * p, :, NMAX:2 * NMAX]], axis=0)
             for p in range(NPAIR)])
        in_maps.append({
            "x": xp[s0:s0 + BL],
            "wt": wt,
            "eadd": np.ascontiguousarray(epair.astype(bf16)),
            "eswp": np.ascontiguousarray(eswp.astype(bf16)),
        })
    res = run_bass_kernel_spmd(nc, in_maps, list(range(N_CORES)))
    global _LAST_RESULTS
    _LAST_RESULTS = res
    out_ev = np.concatenate(
        [res.results[k]["out_ev"] for k in range(N_CORES)], axis=0)
    out_od = np.concatenate(
        [res.results[k]["out_od"] for k in range(N_CORES)], axis=0)
    return _assemble(out_ev, out_od)


_LAST_RESULTS = None  # BassKernelResults of the most recent run (test harness)

